# revision 1
# baseline (speedup 1.0000x reference)
"""Dynamic graph construction (topk mask) Trainium2 Bass kernel.

Math: for each row r of 32 distances (B*T*P rows total), the reference
builds adjacency = one-hot(4 nearest neighbors, diag masked) + eye, then
symmetric-normalizes.  Every row degree is exactly 5 (4 neighbors + self
loop), so the output is simply

    out[r, j] = s * indicator(v[r, j] <= t_r),   s = f32(f32(5^-0.5)^2)

where t_r is the 5th-smallest value of the row (the diagonal zero is
always the smallest, so the 5 smallest = diag + 4 nearest neighbors).

Device algorithm per chunk (graduated sizes, [128 partitions x F rows x 32];
small chunks at the head/tail cut pipeline fill/drain by ~3 us):
  1. SP   : DMA in (512 KiB contiguous per transfer)
  2. ScalarE: w = -v
  3. VectorE: 32x max8 (top-8 per partition row); the 5th largest of w
     per row is the threshold t_w = -t_r
  4. VectorE: mask = (w >= t_w) via one broadcast tensor_tensor is_ge
  5. ScalarE: out = mask * s
  6. SP   : DMA out
All waits are standalone wait_ge instructions (this walrus build accepts
only one sync wait per instruction, which also rules out the Tile
scheduler -- its tail drain carries many waits).  Measured on HW
(K-unrolled NEFF wall-time differencing, bubble-free interleaved
unroll): ~64-67 us steady-state per pass per core; cost-model single
pass ~75 us; HBM roofline (16.8 MB per core
at ~358 GB/s) is ~47 us.  The wall is the DVE: 512 per-row max8 ops
(58-cycle issue overhead each, unavoidable at row length 32) + the
broadcast compare = ~67 us busy.  Rebalancing attempts that did NOT pay:
gpsimd TensorTensor / scalar_tensor_tensor fail walrus engine checks;
gpsimd per-subtile tensor_scalar works but costs ~0.75 us per op
(106 us/pass); ScalarE has no compare/step function.

Sharding: embarrassingly data-parallel; batch axis (64) split across the
8 cores, 8 batches (65536 rows, 8 MiB) per core, no communication.

Tie handling: top_k breaks value-ties by lowest index; a value threshold
selects all tied elements.  Rows whose 5th- and 6th-smallest values tie
(rare: 1 row in 2^19 for this distribution) are canonicalized on the host
by bumping every non-first tied occurrence one ulp up, which makes the
threshold semantics exactly equal to top_k's for any input.
"""

import sys

for _p in ("/opt/trn_rl_repo",):
    if _p not in sys.path:
        sys.path.insert(0, _p)

import numpy as np

from contextlib import ExitStack

import concourse.bass as bass
import concourse.mybir as mybir
from concourse.bass_utils import run_bass_kernel_spmd

N_CORES = 8
B, T, P = 64, 256, 32
B_PER_CORE = B // N_CORES
ROWS_PER_CORE = B_PER_CORE * T * P  # 65536
# Graduated chunk plan (rows-per-partition per chunk): small chunks at the
# head start the DVE sooner; small chunks at the tail drain faster.  Sums
# to ROWS_PER_CORE/128 = 512.
CHUNKS = [4, 12, 24] + [36] * 12 + [20, 8, 8, 4]
NT = len(CHUNKS)
OFFS = [sum(CHUNKS[:c]) for c in range(NT)]
assert sum(CHUNKS) == ROWS_PER_CORE // 128

# s = f32(c*c), c = f32(5**-0.5): matches reference's dinv_i * A * dinv_j
_C = np.float32(5.0) ** np.float32(-0.5)
SCALE = float(np.float32(_C * _C))

_compiled = None


def _build_bass(iters=1):
    nc = bass.Bass("TRN2", target_bir_lowering=False, debug=False,
                   num_devices=N_CORES)
    x = nc.declare_dram_parameter("x", [ROWS_PER_CORE, P], mybir.dt.float32,
                                  isOutput=False)
    y = nc.declare_dram_parameter("y", [ROWS_PER_CORE, P], mybir.dt.float32,
                                  isOutput=True)

    # Partition-major layout: partition p owns rows [p*512, (p+1)*512);
    # chunk c covers the per-partition sub-range [OFFS[c], OFFS[c]+CHUNKS[c])
    xg = x.rearrange("(p f) c -> p (f c)", p=128)
    yg = y.rearrange("(p f) c -> p (f c)", p=128)
    xv = [xg[:, OFFS[c] * P:(OFFS[c] + CHUNKS[c]) * P] for c in range(NT)]
    yv = [yg[:, OFFS[c] * P:(OFFS[c] + CHUNKS[c]) * P] for c in range(NT)]

    # Raw bass (no Tile): this walrus toolchain only accepts ONE sync wait
    # per instruction, so all waits are standalone wait_ge ops.  Unique
    # buffers per chunk, single writer each; per-engine streams pipeline
    # naturally (DMA-in i+1 overlaps compute i overlaps DMA-out i-1).
    with ExitStack() as ctx:
        vs = [ctx.enter_context(
                  nc.sbuf_tensor(f"v{i}", [128, CHUNKS[i] * P],
                                 mybir.dt.float32))
              for i in range(NT)]
        ws = [ctx.enter_context(
                  nc.sbuf_tensor(f"w{i}", [128, CHUNKS[i] * P],
                                 mybir.dt.float32))
              for i in range(NT)]
        m8s = [ctx.enter_context(
                   nc.sbuf_tensor(f"m8{i}", [128, CHUNKS[i] * 8],
                                  mybir.dt.float32))
               for i in range(NT)]
        # One DMA-in sem PER CHUNK: a single shared counter is unsafe
        # because HWDGE completions can reorder across in-flight DMAs
        # (observed: first-exec corruption at chunk starts when a consumer
        # unblocked on a later chunk's completion).
        in_sems = [ctx.enter_context(nc.semaphore(f"in_sem{i}"))
                   for i in range(NT)]
        neg_sem = ctx.enter_context(nc.semaphore("neg_sem"))
        pl_sem = ctx.enter_context(nc.semaphore("pl_sem"))
        scl_sem = ctx.enter_context(nc.semaphore("scl_sem"))
        out_sem = ctx.enter_context(nc.semaphore("out_sem"))

        sems = (*in_sems, neg_sem, pl_sem, scl_sem, out_sem)
        ids = sorted(s.num for s in sems)
        sem_range = range(ids[0], ids[-1] + 1)

        # Pipeline per chunk i (of NT), iteration k (benchmark unroll):
        #   SP  : DMA-in x -> v[i]                 inc in_sems[i]+16
        #   ACT : w[i] = -v[i]                     inc neg_sem
        #   DVE : m8[i] = per-row top8(w[i]),
        #         w[i] = (w[i] >= m8[i][...,4])    inc pl_sem
        #   ACT : w[i] *= s                        inc scl_sem
        #   SP  : DMA-out w[i] -> y                inc out_sem+16
        with nc.Block() as block:

            @block.sync
            def _(sync):
                # iteration-k+1 loads interleave with iteration-k stores so
                # the unrolled benchmark has no per-iteration bubble; for
                # iters=1 this emits exactly loads-then-stores.
                for i in range(NT):
                    sync.dma_start(out=vs[i][:, :],
                                   in_=xv[i]).then_inc(in_sems[i], 16)
                for k in range(iters):
                    for i in range(NT):
                        sync.wait_ge(scl_sem, k * NT + i + 1)
                        sync.dma_start(out=yv[i],
                                       in_=ws[i][:, :]).then_inc(out_sem, 16)
                        if k + 1 < iters:
                            # WAR: v[i] free once ACT(k,i) has read it
                            sync.wait_ge(neg_sem, k * NT + i + 1)
                            sync.dma_start(out=vs[i][:, :],
                                           in_=xv[i]).then_inc(in_sems[i], 16)
                sync.wait_ge(out_sem, iters * NT * 16)

            @block.scalar
            def _(scalar):
                for k in range(iters):
                    for i in range(NT):
                        scalar.wait_ge(in_sems[i], 16 * (k + 1))
                        if k > 0:  # WAR: w[i] read by out-DMA of k-1
                            scalar.wait_ge(out_sem, 16 * ((k - 1) * NT + i + 1))
                        scalar.activation(
                            out=ws[i][:, :], in_=vs[i][:, :],
                            func=mybir.ActivationFunctionType.Copy,
                            scale=-1.0).then_inc(neg_sem, 1)
                    for i in range(NT):
                        scalar.wait_ge(pl_sem, k * NT + i + 1)
                        scalar.activation(
                            out=ws[i][:, :], in_=ws[i][:, :],
                            func=mybir.ActivationFunctionType.Copy,
                            scale=SCALE).then_inc(scl_sem, 1)

            @block.vector
            def _(vector):
                for k in range(iters):
                    for i in range(NT):
                        Fi = CHUNKS[i]
                        vector.wait_ge(neg_sem, k * NT + i + 1)
                        w3 = ws[i].rearrange("p (f c) -> p f c", c=P)
                        m8 = m8s[i]
                        for s in range(Fi):
                            vector.max(out=m8[:, s * 8:(s + 1) * 8],
                                       in_=w3[:, s, :])
                        t_b = (m8.rearrange("p (f e) -> p f e", e=8)[:, :, 4:5]
                               .to_broadcast([128, Fi, P]))
                        vector.tensor_tensor(
                            out=w3, in0=w3, in1=t_b,
                            op=mybir.AluOpType.is_ge).then_inc(pl_sem, 1)

        # NEFFs execute repeatedly and our wait targets are absolute, so
        # every run must start with zeroed semaphores.  Clearing at kernel
        # START races the other engines' first increments (observed
        # first-exec corruption: a cold gpsimd clears late and wipes live
        # counts).  Instead clear at the END, after the Block-exit
        # all-engine barrier proves every wait has already passed; NRT
        # zero-initializes the very first run after load.
        nc.gpsimd.sem_clear(sem_range)
    return nc


def _canonicalize_ties(flat):
    """Bump non-first occurrences of each row's 5th-smallest value by one
    ulp when the 5th and 6th smallest tie, so that (v <= t5) selects
    exactly the 5 elements jax.lax.top_k would (lowest index first)."""
    part = np.partition(flat, (4, 5), axis=1)
    bad = np.nonzero(part[:, 4] == part[:, 5])[0]
    if len(bad) == 0:
        return flat
    flat = flat.copy()
    for r in bad:
        t = part[r, 4]
        row = flat[r]
        n_less = int((row < t).sum())
        keep = 5 - n_less  # tied occurrences top_k keeps, in index order
        idx = np.nonzero(row == t)[0]
        row[idx[keep:]] = np.nextafter(t, np.float32(np.inf), dtype=np.float32)
    return flat


def kernel(distances: np.ndarray) -> np.ndarray:
    global _compiled
    assert distances.shape == (B, T, P, P) and distances.dtype == np.float32

    flat = _canonicalize_ties(
        np.ascontiguousarray(distances).reshape(-1, P))

    if _compiled is None:
        _compiled = _build_bass()
    nc = _compiled

    shards = np.split(flat.reshape(N_CORES, ROWS_PER_CORE, P), N_CORES, axis=0)
    in_maps = [{"x": np.ascontiguousarray(s[0])} for s in shards]
    res = run_bass_kernel_spmd(nc, in_maps, list(range(N_CORES)))
    outs = [res.results[i]["y"].reshape(B_PER_CORE, T, P, P)
            for i in range(N_CORES)]
    return np.concatenate(outs, axis=0)


if __name__ == "__main__":
    d = np.load("/root/problem/distances.npy")
    out = kernel(distances=d)
    exp = np.load("/root/problem/expected.npy")
    err = np.abs(out - exp)
    print("max abs err:", err.max(), "mismatches:", int((err > 1e-6).sum()))

